# revision 39
# baseline (speedup 1.0000x reference)
"""Trainium2 Bass kernel for GNN attention message passing.

Reference computation (per query node b, step s, neighbors k=0..31):
    scores[s,b,k] = ne[s,b,k] . w_nb + node_e[b] . w_self + fc_b
    attn = softmax_k(leaky_relu(scores, 0.2))
    out[b] = sum_{s,k} attn[s,b,k] * ne[s,b,k] + S*K * node_e[b]

Sharding: data-parallel over the node batch B=4096 across 8 cores (512
query nodes per core).  Each core receives a compacted bf16 embedding
table holding each row it needs exactly once (host-side np.unique remap
so indices fit int16 for the on-device dma_gather) and gathers all
2*512*32 = 32768 neighbor rows on device.

Per-core pipeline (per 4096-row chunk, 8 chunks):
  * mixed-mode dma_gather: the 4 x 1024-row sub-gathers alternate
    per-descriptor-packet mode (cheap Q7 descriptor generation, drains
    on the 4 queue-bound DMA engines) and single-packet mode (pricier
    generation, drains across all 16 DMA engines), rotating over the 4
    SWDGE queues -- balancing the two per-descriptor bottlenecks gives
    ~120us for the gather stream vs ~206us for either mode alone
  * scores: fused multiply + free-axis-reduce (scalar_tensor_tensor
    with accum_out) on the vector engine, one op per 128-row tile
  * softmax runs in a transposed layout (TensorE transpose puts the
    tile index on partitions, neighbor index on the free axis) so the
    k=32 segments reduce on the free axis; fc_b + the node-term bias
    fold into one scalar_tensor_tensor; exp on the scalar engine
  * aggregation: block-diagonal M=32 matmuls on TensorE (stationary =
    position-mask * attn), accumulating both steps in 4 PSUM banks;
    epilogue adds (S*K) * node_e and streams results out

All engines overlap; measured ~157us/8-core-chip, rel err ~1.7e-3
(bf16 storage, fp32 accumulation).  KERNEL_DT=f32 gives an exact
(3e-8) fallback at ~300us.
"""

import os
import sys

for _p in ("/opt/trn_rl_repo", "/root/.axon_site/_ro/trn_rl_repo"):
    if os.path.isdir(_p) and _p not in sys.path:
        sys.path.insert(0, _p)

import numpy as np

import concourse.bass as bass
import concourse.bacc as bacc
import concourse.tile as tile
from concourse import mybir
from concourse.bass_utils import run_bass_kernel_spmd

# Problem constants (hardcoded per spec)
N_NODES = 100000
D = 256
STEPS = 2
K = 32
B = 4096
NEG_SLOPE = 0.2
N_CORES = 8

B_LOC = B // N_CORES  # 512 query nodes per core
ROWS = STEPS * B_LOC * K  # 32768 gathered neighbor rows per core
TILES = ROWS // 128  # 256
CHUNK_TILES = 32  # tiles per gather chunk
CHUNK_ROWS = CHUNK_TILES * 128  # 4096
N_CHUNKS = TILES // CHUNK_TILES  # 8
U_PAD = 32768  # compacted table rows (padded, fits int16 indexing)

# bf16 storage for the embedding table halves HBM traffic and doubles
# vector-engine throughput; fp32 accumulation throughout keeps the
# result well inside the 2e-2 relative-error gate.
DT_NAME = os.environ.get("KERNEL_DT", "bf16")

_CACHE = {}


def _np_dt(dt_name):
    if dt_name == "bf16":
        import ml_dtypes

        return np.dtype(ml_dtypes.bfloat16)
    return np.dtype(np.float32)


STAGE = int(os.environ.get("KERNEL_STAGE", "9"))  # 1=scores 2=softmax 9=full


def _build_nc(dt_name, fc_w, fc_b):
    """Build the per-core Bass graph (same NEFF for all 8 cores)."""
    DT = mybir.dt.bfloat16 if dt_name == "bf16" else mybir.dt.float32
    F32 = mybir.dt.float32
    npdt = _np_dt(dt_name)

    nc = bacc.Bacc(num_swdge_queues=4)

    table = nc.dram_tensor("table", [U_PAD, D], DT, kind="ExternalInput")
    neidx = nc.dram_tensor(
        "neidx", [128, ROWS // 16], mybir.dt.int16, kind="ExternalInput"
    )
    ndidx = nc.dram_tensor("ndidx", [128, 64], mybir.dt.int16, kind="ExternalInput")
    out_d = nc.dram_tensor("out", [B_LOC, D], F32, kind="ExternalOutput")

    w_nb = np.asarray(fc_w[0, :D], dtype=np.float32)
    w_self = np.asarray(fc_w[0, D:], dtype=np.float32)
    fcb = float(np.asarray(fc_b).reshape(-1)[0])

    wnb_c = nc.inline_tensor(
        np.tile(w_nb[None, :], (128, 1)).astype(npdt), name="wnb_c"
    )
    wself_c = nc.inline_tensor(
        np.tile(w_self[None, :], (128, 1)).astype(npdt), name="wself_c"
    )
    # mask8[p, q, m] = 1 iff m == 4q + p//32: selects the output column for
    # a tile at position q (of 8) within a 32-b output quarter
    mask8_np = np.zeros((128, 8, 32), dtype=np.float32)
    for p in range(128):
        for q in range(8):
            mask8_np[p, q, 4 * q + p // 32] = 1.0
    mask_c = nc.inline_tensor(mask8_np.astype(npdt), name="mask_c")
    ident_c = nc.inline_tensor(np.eye(128, dtype=np.float32), name="ident_c")

    with tile.TileContext(nc) as tc:
        with (
            tc.tile_pool(name="consts", bufs=1) as consts,
            tc.tile_pool(name="idxp", bufs=1) as idxp,
            tc.tile_pool(name="nep", bufs=4 if dt_name == "bf16" else 2) as nep,
            tc.tile_pool(name="prodp", bufs=6) as prodp,
            tc.tile_pool(name="scorep", bufs=1) as scorep,
            tc.tile_pool(name="smaxp", bufs=2) as smaxp,
            tc.tile_pool(name="outp", bufs=2) as outp,
            tc.tile_pool(name="psum_t", bufs=2, space="PSUM") as psum_t,
            tc.tile_pool(name="psum_agg", bufs=1, space="PSUM") as psum_agg,
        ):
            # ---- index tensors first (the chunk-0 gather is the critical path) ----
            neidx_sb = idxp.tile([128, ROWS // 16], mybir.dt.int16, tag="neidx")
            _slot = CHUNK_ROWS // 16
            nc.sync.dma_start(out=neidx_sb[:, 0:_slot], in_=neidx[:, 0:_slot])
            ndidx_sb = idxp.tile([128, 64], mybir.dt.int16, tag="ndidx")
            nc.sync.dma_start(out=ndidx_sb[:], in_=ndidx[:])
            for _c in range(1, N_CHUNKS):
                nc.sync.dma_start(
                    out=neidx_sb[:, _c * _slot : (_c + 1) * _slot],
                    in_=neidx[:, _c * _slot : (_c + 1) * _slot],
                )

            # ---- constants to SBUF (ACT HWDGE ring; not on the gather critical path) ----
            wnb_sb = consts.tile([128, D], DT, tag="wnb")
            nc.scalar.dma_start(out=wnb_sb[:], in_=wnb_c[:])
            wself_sb = consts.tile([128, D], DT, tag="wself")
            nc.scalar.dma_start(out=wself_sb[:], in_=wself_c[:])
            mask_sb = consts.tile([128, 8, 32], DT, tag="mask")
            nc.scalar.dma_start(out=mask_sb[:], in_=mask_c[:])
            ident_sb = consts.tile([128, 128], F32, tag="ident")
            nc.scalar.dma_start(out=ident_sb[:], in_=ident_c[:])

            s_all = scorep.tile([128, TILES], F32, tag="s_all")
            _gq = [0]
            node_sb = consts.tile([128, 8, D], DT, tag="node_sb")
            c_T0 = consts.tile([128, 4], F32, tag="c_T0")

            for c in range(N_CHUNKS):
                jb = c % 4
                # ---- gather 4096 neighbor embedding rows ----
                nslots = CHUNK_ROWS // 16
                nsub = 4
                stiles = CHUNK_TILES // nsub
                ne_subs = [
                    nep.tile(
                        [128, stiles, D], DT,
                        tag=f"ne{s}", name=f"ne_c{c}s{s}",
                    )
                    for s in range(nsub)
                ]

                def ne_tile(i, _subs=ne_subs, _st=stiles):
                    return _subs[i // _st][:, i % _st, :]

                # mixed-mode gather: sub-gather 0 uses per-descriptor packets
                # (cheap descriptor generation, drains on the 4 queue-bound
                # engines); sub-gathers 1-3 use single-packet mode (pricier
                # generation, drains across all 16 DMA engines).  Interleaving
                # the two balances the Q7 generation and engine-drain limits.
                for s in range(nsub):
                    sr = CHUNK_ROWS // nsub
                    ss = nslots // nsub
                    if c == 0 and s == 0:
                        # node-embedding rows first (small; unblocks the score
                        # bias c_T0), then chunk 0's first neighbor sub-gather
                        # single-packet: spreads across all 16 DMA engines so
                        # the very first tiles' data lands as early as possible
                        nc.gpsimd.dma_gather(
                            out_ap=ne_subs[0][:],
                            in_ap=table[:],
                            idxs_ap=neidx_sb[:, 0:ss],
                            num_idxs=sr,
                            num_idxs_reg=sr,
                            elem_size=D,
                            single_packet=True,
                            queue_num=0,
                        )
                        nc.gpsimd.dma_gather(
                            out_ap=node_sb[:],
                            in_ap=table[:],
                            idxs_ap=ndidx_sb[:],
                            num_idxs=2 * B_LOC,
                            num_idxs_reg=2 * B_LOC,
                            elem_size=D,
                            single_packet=False,
                            queue_num=1,
                        )
                        _gq[0] += 2
                        continue
                    nc.gpsimd.dma_gather(
                        out_ap=ne_subs[s][:],
                        in_ap=table[:],
                        idxs_ap=neidx_sb[:, c * nslots + s * ss : c * nslots + (s + 1) * ss],
                        num_idxs=sr,
                        num_idxs_reg=sr,
                        elem_size=D,
                        single_packet=(c == 0 or s != 0),
                        queue_num=_gq[0] % 4,
                    )
                    _gq[0] += 1

                # ---- scores: bf16 multiply on DVE (2x perf mode), then the
                # free-axis sum via the scalar engine's Copy+accumulator --
                # splitting the fused op across two engines halves the
                # vector-engine cost per tile ----
                for i in range(CHUNK_TILES):
                    prod = prodp.tile([128, D], DT, tag="prod")
                    nc.vector.tensor_tensor(
                        out=prod[:],
                        in0=ne_tile(i),
                        in1=wnb_sb[:],
                        op=mybir.AluOpType.mult,
                    )
                    sink = prodp.tile([128, D], DT, tag="sink")
                    nc.scalar.activation(
                        out=sink[:],
                        in_=prod[:],
                        func=mybir.ActivationFunctionType.Copy,
                        accum_out=s_all[:, c * CHUNK_TILES + i : c * CHUNK_TILES + i + 1],
                    )

                if STAGE < 2:
                    if c == N_CHUNKS - 1:
                        nc.sync.dma_start(out=out_d[0:128, :], in_=s_all[:])
                    continue

                if c == 0:
                    # c_T0[j, g] = node_e[4j+g] . w_self  (fc_b folded into u)
                    for g in range(4):
                        prod = prodp.tile([128, D], DT, tag="prod")
                        nc.vector.scalar_tensor_tensor(
                            out=prod[:],
                            in0=node_sb[:, g, :],
                            scalar=1.0,
                            in1=wself_sb[:],
                            op0=mybir.AluOpType.mult,
                            op1=mybir.AluOpType.mult,
                            accum_out=c_T0[:, g : g + 1],
                        )

                # ---- transpose scores: [128, 32] -> [32, 128] ----
                sT_ps = psum_t.tile([32, 128], F32, tag="sT")
                nc.tensor.transpose(
                    out=sT_ps[:],
                    in_=s_all[:, c * CHUNK_TILES : (c + 1) * CHUNK_TILES],
                    identity=ident_sb[:],
                )

                # ---- softmax over k in transposed layout ----
                # u = scores_T + c_T (bias constant over k, varies per group)
                cslice = c_T0[32 * jb : 32 * jb + 32, :]
                u = smaxp.tile([32, 128], F32, tag="u")
                nc.vector.scalar_tensor_tensor(
                    out=u[:].rearrange("p (g k) -> p g k", g=4),
                    in0=sT_ps[:].rearrange("p (g k) -> p g k", g=4),
                    scalar=fcb,
                    in1=cslice.to_broadcast([32, 4, K]),
                    op0=mybir.AluOpType.add,
                    op1=mybir.AluOpType.add,
                )
                # leaky_relu(u) = max(0.2*u, u)
                lr = smaxp.tile([32, 128], F32, tag="lr")
                nc.vector.scalar_tensor_tensor(
                    out=lr[:],
                    in0=u[:],
                    scalar=NEG_SLOPE,
                    in1=u[:],
                    op0=mybir.AluOpType.mult,
                    op1=mybir.AluOpType.max,
                )
                ex = smaxp.tile([32, 128], F32, tag="ex")
                nc.scalar.activation(
                    out=ex[:],
                    in_=lr[:],
                    func=mybir.ActivationFunctionType.Exp,
                )
                dn = smaxp.tile([32, 4], F32, tag="dn")
                nc.vector.tensor_reduce(
                    out=dn[:],
                    in_=ex[:].rearrange("p (g k) -> p g k", g=4),
                    axis=mybir.AxisListType.X,
                    op=mybir.AluOpType.add,
                )
                rcp = smaxp.tile([32, 4], F32, tag="rcp")
                nc.vector.reciprocal(out=rcp[:], in_=dn[:])
                attn_T = smaxp.tile([32, 128], F32, tag="attn_T")
                attn_eng = nc.vector
                attn_eng.tensor_tensor(
                    out=attn_T[:].rearrange("p (g k) -> p g k", g=4),
                    in0=ex[:].rearrange("p (g k) -> p g k", g=4),
                    in1=rcp[:].to_broadcast([32, 4, K]),
                    op=mybir.AluOpType.mult,
                )

                # ---- transpose back: [32, 128] -> [128, 32] ----
                attn_ps = psum_t.tile([128, 32], F32, tag="attn_ps")
                nc.tensor.transpose(
                    out=attn_ps[:],
                    in_=attn_T[:],
                    identity=ident_sb[0:32, 0:32],
                )

                # ---- stationary matrices: am[p, j//8, j%8, m] =
                #      mask8[p, j%8, m] * attn[p, j] ----
                attn_sb = smaxp.tile([128, CHUNK_TILES], DT, tag="attn_sb")
                nc.scalar.copy(out=attn_sb[:], in_=attn_ps[:])
                am = smaxp.tile([128, 4, 8, 32], DT, tag="am")
                m_ap = mask_sb[:]
                mask_bc = bass.AP(
                    tensor=m_ap.tensor,
                    offset=m_ap.offset,
                    ap=[m_ap.ap[0], [0, 4], m_ap.ap[1], m_ap.ap[2]],
                )
                a_ap = attn_sb[:]
                attn_bc = bass.AP(
                    tensor=a_ap.tensor,
                    offset=a_ap.offset,
                    ap=[a_ap.ap[0], [8 * a_ap.ap[1][0], 4], [a_ap.ap[1][0], 8], [0, 32]],
                )
                am_eng = nc.gpsimd if c == 6 else nc.vector
                am_eng.tensor_tensor(
                    out=am[:],
                    in0=mask_bc,
                    in1=attn_bc,
                    op=mybir.AluOpType.mult,
                )

                if STAGE < 3:
                    if c == 0:
                        o32 = outp.tile([128, 32], F32, tag="o32")
                        nc.vector.tensor_copy(out=o32[:], in_=attn_sb[:])
                        nc.sync.dma_start(out=out_d[0:128, 0:32], in_=o32[:])
                    continue

                # ---- block-diagonal aggregation matmuls (M=32, 32-aligned) ----
                if c < 4:
                    agg = psum_agg.tile([128, D], F32, tag=f"agg{jb}")
                    _CACHE.setdefault("agg_tiles", {})[jb] = agg
                else:
                    agg = _CACHE["agg_tiles"][jb]
                for j in range(CHUNK_TILES):
                    qpos = 32 * (j // 8)
                    nc.tensor.matmul(
                        out=agg[qpos : qpos + 32, :],
                        lhsT=am[:, j // 8, j % 8, :],
                        rhs=ne_tile(j),
                        start=(c < 4 and j % 8 == 0),
                        stop=(c >= 4 and j % 8 == 7),
                        skip_group_check=True,
                        tile_position=(0, qpos),
                    )

                # ---- epilogue: out = agg + (S*K) * node_e ----
                if c >= 4:
                    o_sb = outp.tile([128, D], F32, tag="o_sb")
                    nc.vector.scalar_tensor_tensor(
                        out=o_sb[:],
                        in0=node_sb[:, 4 + jb, :],
                        scalar=float(STEPS * K),
                        in1=agg[:],
                        op0=mybir.AluOpType.mult,
                        op1=mybir.AluOpType.add,
                    )
                    nc.sync.dma_start(
                        out=out_d[128 * jb : 128 * (jb + 1), :], in_=o_sb[:]
                    )

    nc.compile()
    _CACHE.pop("agg_tiles", None)
    return nc


def _prep_core_inputs(core, node, neighbors, embeddings, npdt):
    """Host-side sharding: compact the table and remap indices (int16)."""
    node_c = np.asarray(node[B_LOC * core : B_LOC * (core + 1)])
    nb_c = np.asarray(neighbors[:, node_c, :])  # [S, B_LOC, K]
    flat = nb_c.reshape(-1).astype(np.int64)  # row r = s*B_LOC*K + b*K + k
    allidx = np.concatenate([flat, node_c.astype(np.int64)])
    uniq, inv = np.unique(allidx, return_inverse=True)
    U = len(uniq)
    assert U <= U_PAD, f"core {core}: {U} unique rows exceed {U_PAD}"
    tbl = np.zeros((U_PAD, D), dtype=npdt)
    tbl[:U] = embeddings[uniq].astype(npdt)

    flat16 = inv[:ROWS].astype(np.int16)
    node16 = inv[ROWS:].astype(np.int16)

    # neighbor indices, wrapped per chunk: index q of chunk c sits at
    # [partition q%16 (replicated x8), slot c*256 + q//16]
    ne_w = np.zeros((128, ROWS // 16), dtype=np.int16)
    for c in range(N_CHUNKS):
        chunk = flat16[CHUNK_ROWS * c : CHUNK_ROWS * (c + 1)]
        wrapped = chunk.reshape(CHUNK_ROWS // 16, 16).T  # [16, 256]
        ne_w[:, (ROWS // 16 // N_CHUNKS) * c : (ROWS // 16 // N_CHUNKS) * (c + 1)] = (
            np.tile(wrapped, (8, 1))
        )

    # node gathers: c-order (gathered row i -> node[4*(i%128) + i//128]),
    # then natural order
    i = np.arange(B_LOC)
    cidx = node16[4 * (i % 128) + i // 128]
    nd = np.concatenate([cidx, node16])  # 1024 indices
    nd_w = np.tile(nd.reshape(64, 16).T, (8, 1)).astype(np.int16)  # [128, 64]

    return {"table": tbl, "neidx": ne_w, "ndidx": nd_w}


def kernel(node, neighbors, embeddings, fc_w, fc_b, _trace=False):
    node = np.asarray(node)
    neighbors = np.asarray(neighbors)
    embeddings = np.asarray(embeddings, dtype=np.float32)
    fc_w = np.asarray(fc_w, dtype=np.float32)
    fc_b = np.asarray(fc_b, dtype=np.float32)

    npdt = _np_dt(DT_NAME)
    key = (DT_NAME, fc_w.tobytes(), fc_b.tobytes())
    if _CACHE.get("key") != key:
        _CACHE["nc"] = _build_nc(DT_NAME, fc_w, fc_b)
        _CACHE["key"] = key
    nc = _CACHE["nc"]

    in_maps = [
        _prep_core_inputs(c, node, neighbors, embeddings, npdt)
        for c in range(N_CORES)
    ]
    res = run_bass_kernel_spmd(
        nc, in_maps, core_ids=list(range(N_CORES)), trace=_trace
    )
    out = np.concatenate([res.results[c]["out"] for c in range(N_CORES)], axis=0)
    if _trace:
        _CACHE["last_exec_time_ns"] = res.exec_time_ns
        _CACHE["last_results"] = res
    return out


# revision 40
# speedup vs baseline: 1.2810x; 1.2810x over previous
"""Trainium2 Bass kernel for GNN attention message passing.

Reference computation (per query node b, step s, neighbors k=0..31):
    scores[s,b,k] = ne[s,b,k] . w_nb + node_e[b] . w_self + fc_b
    attn = softmax_k(leaky_relu(scores, 0.2))
    out[b] = sum_{s,k} attn[s,b,k] * ne[s,b,k] + S*K * node_e[b]

Sharding: data-parallel over the node batch B=4096 across 8 cores (512
query nodes per core).  Each core receives a compacted bf16 embedding
table holding each row it needs exactly once (host-side np.unique remap
so indices fit int16 for the on-device dma_gather) and gathers all
2*512*32 = 32768 neighbor rows on device.

Per-core pipeline (per 4096-row chunk, 8 chunks):
  * mixed-mode dma_gather: the 4 x 1024-row sub-gathers alternate
    per-descriptor-packet mode (cheap Q7 descriptor generation, drains
    on the 4 queue-bound DMA engines) and single-packet mode (pricier
    generation, drains across all 16 DMA engines), rotating over the 4
    SWDGE queues -- balancing the two per-descriptor bottlenecks gives
    ~120us for the gather stream vs ~206us for either mode alone
  * scores: fused multiply + free-axis-reduce (scalar_tensor_tensor
    with accum_out) on the vector engine, one op per 128-row tile
  * softmax runs in a transposed layout (TensorE transpose puts the
    tile index on partitions, neighbor index on the free axis) so the
    k=32 segments reduce on the free axis; fc_b + the node-term bias
    fold into one scalar_tensor_tensor; exp on the scalar engine
  * aggregation: block-diagonal M=32 matmuls on TensorE (stationary =
    position-mask * attn), accumulating both steps in 4 PSUM banks;
    epilogue adds (S*K) * node_e and streams results out

All engines overlap; measured ~157us/8-core-chip, rel err ~1.7e-3
(bf16 storage, fp32 accumulation).  KERNEL_DT=f32 gives an exact
(3e-8) fallback at ~300us.
"""

import os
import sys

for _p in ("/opt/trn_rl_repo", "/root/.axon_site/_ro/trn_rl_repo"):
    if os.path.isdir(_p) and _p not in sys.path:
        sys.path.insert(0, _p)

import numpy as np

import concourse.bass as bass
import concourse.bacc as bacc
import concourse.tile as tile
from concourse import mybir
from concourse.bass_utils import run_bass_kernel_spmd

# Problem constants (hardcoded per spec)
N_NODES = 100000
D = 256
STEPS = 2
K = 32
B = 4096
NEG_SLOPE = 0.2
N_CORES = 8

B_LOC = B // N_CORES  # 512 query nodes per core
ROWS = STEPS * B_LOC * K  # 32768 gathered neighbor rows per core
TILES = ROWS // 128  # 256
CHUNK_TILES = 32  # tiles per gather chunk
CHUNK_ROWS = CHUNK_TILES * 128  # 4096
N_CHUNKS = TILES // CHUNK_TILES  # 8
U_PAD = 32768  # compacted table rows (padded, fits int16 indexing)

# bf16 storage for the embedding table halves HBM traffic and doubles
# vector-engine throughput; fp32 accumulation throughout keeps the
# result well inside the 2e-2 relative-error gate.
DT_NAME = os.environ.get("KERNEL_DT", "bf16")

_CACHE = {}


def _np_dt(dt_name):
    if dt_name == "bf16":
        import ml_dtypes

        return np.dtype(ml_dtypes.bfloat16)
    return np.dtype(np.float32)


STAGE = int(os.environ.get("KERNEL_STAGE", "9"))  # 1=scores 2=softmax 9=full


def _build_nc(dt_name, fc_w, fc_b):
    """Build the per-core Bass graph (same NEFF for all 8 cores)."""
    DT = mybir.dt.bfloat16 if dt_name == "bf16" else mybir.dt.float32
    F32 = mybir.dt.float32
    npdt = _np_dt(dt_name)

    nc = bacc.Bacc(num_swdge_queues=4)

    table = nc.dram_tensor("table", [U_PAD, D], DT, kind="ExternalInput")
    neidx = nc.dram_tensor(
        "neidx", [128, ROWS // 16], mybir.dt.int16, kind="ExternalInput"
    )
    ndidx = nc.dram_tensor("ndidx", [128, 64], mybir.dt.int16, kind="ExternalInput")
    out_d = nc.dram_tensor("out", [B_LOC, D], F32, kind="ExternalOutput")

    w_nb = np.asarray(fc_w[0, :D], dtype=np.float32)
    w_self = np.asarray(fc_w[0, D:], dtype=np.float32)
    fcb = float(np.asarray(fc_b).reshape(-1)[0])

    wnb_c = nc.inline_tensor(
        np.tile(w_nb[None, :], (128, 1)).astype(npdt), name="wnb_c"
    )
    wself_c = nc.inline_tensor(
        np.tile(w_self[None, :], (128, 1)).astype(npdt), name="wself_c"
    )
    # mask8[p, q, m] = 1 iff m == 4q + p//32: selects the output column for
    # a tile at position q (of 8) within a 32-b output quarter
    mask8_np = np.zeros((128, 8, 32), dtype=np.float32)
    for p in range(128):
        for q in range(8):
            mask8_np[p, q, 4 * q + p // 32] = 1.0
    mask_c = nc.inline_tensor(mask8_np.astype(npdt), name="mask_c")
    ident_c = nc.inline_tensor(np.eye(128, dtype=np.float32), name="ident_c")

    with tile.TileContext(nc) as tc:
        with (
            tc.tile_pool(name="consts", bufs=1) as consts,
            tc.tile_pool(name="idxp", bufs=1) as idxp,
            tc.tile_pool(name="nep", bufs=4 if dt_name == "bf16" else 2) as nep,
            tc.tile_pool(name="prodp", bufs=6) as prodp,
            tc.tile_pool(name="scorep", bufs=1) as scorep,
            tc.tile_pool(name="smaxp", bufs=2) as smaxp,
            tc.tile_pool(name="outp", bufs=2) as outp,
            tc.tile_pool(name="psum_t", bufs=2, space="PSUM") as psum_t,
            tc.tile_pool(name="psum_agg", bufs=1, space="PSUM") as psum_agg,
        ):
            # ---- index tensors first (the chunk-0 gather is the critical path) ----
            neidx_sb = idxp.tile([128, ROWS // 16], mybir.dt.int16, tag="neidx")
            _slot = CHUNK_ROWS // 16
            nc.sync.dma_start(out=neidx_sb[:, 0:_slot], in_=neidx[:, 0:_slot])
            ndidx_sb = idxp.tile([128, 64], mybir.dt.int16, tag="ndidx")
            nc.sync.dma_start(out=ndidx_sb[:], in_=ndidx[:])
            for _c in range(1, N_CHUNKS):
                nc.sync.dma_start(
                    out=neidx_sb[:, _c * _slot : (_c + 1) * _slot],
                    in_=neidx[:, _c * _slot : (_c + 1) * _slot],
                )

            # ---- constants to SBUF (ACT HWDGE ring; not on the gather critical path) ----
            wnb_sb = consts.tile([128, D], DT, tag="wnb")
            nc.scalar.dma_start(out=wnb_sb[:], in_=wnb_c[:])
            wself_sb = consts.tile([128, D], DT, tag="wself")
            nc.scalar.dma_start(out=wself_sb[:], in_=wself_c[:])
            mask_sb = consts.tile([128, 8, 32], DT, tag="mask")
            nc.scalar.dma_start(out=mask_sb[:], in_=mask_c[:])
            ident_sb = consts.tile([128, 128], F32, tag="ident")
            nc.scalar.dma_start(out=ident_sb[:], in_=ident_c[:])

            s_all = scorep.tile([128, TILES], F32, tag="s_all")
            _gq = [0]
            node_sb = consts.tile([128, 8, D], DT, tag="node_sb")
            c_T0 = consts.tile([128, 4], F32, tag="c_T0")

            for c in range(N_CHUNKS):
                jb = c % 4
                # ---- gather 4096 neighbor embedding rows ----
                nslots = CHUNK_ROWS // 16
                nsub = 4
                stiles = CHUNK_TILES // nsub
                ne_subs = [
                    nep.tile(
                        [128, stiles, D], DT,
                        tag=f"ne{s}", name=f"ne_c{c}s{s}",
                    )
                    for s in range(nsub)
                ]

                def ne_tile(i, _subs=ne_subs, _st=stiles):
                    return _subs[i // _st][:, i % _st, :]

                # mixed-mode gather: sub-gather 0 uses per-descriptor packets
                # (cheap descriptor generation, drains on the 4 queue-bound
                # engines); sub-gathers 1-3 use single-packet mode (pricier
                # generation, drains across all 16 DMA engines).  Interleaving
                # the two balances the Q7 generation and engine-drain limits.
                for s in range(nsub):
                    sr = CHUNK_ROWS // nsub
                    ss = nslots // nsub
                    if c == 0 and s == 0:
                        # node-embedding rows first (small; unblocks the score
                        # bias c_T0), then chunk 0's first neighbor sub-gather
                        # single-packet: spreads across all 16 DMA engines so
                        # the very first tiles' data lands as early as possible
                        nc.gpsimd.dma_gather(
                            out_ap=ne_subs[0][:],
                            in_ap=table[:],
                            idxs_ap=neidx_sb[:, 0:ss],
                            num_idxs=sr,
                            num_idxs_reg=sr,
                            elem_size=D,
                            single_packet=True,
                            queue_num=0,
                        )
                        nc.gpsimd.dma_gather(
                            out_ap=node_sb[:],
                            in_ap=table[:],
                            idxs_ap=ndidx_sb[:],
                            num_idxs=2 * B_LOC,
                            num_idxs_reg=2 * B_LOC,
                            elem_size=D,
                            single_packet=False,
                            queue_num=1,
                        )
                        _gq[0] += 2
                        continue
                    nc.gpsimd.dma_gather(
                        out_ap=ne_subs[s][:],
                        in_ap=table[:],
                        idxs_ap=neidx_sb[:, c * nslots + s * ss : c * nslots + (s + 1) * ss],
                        num_idxs=sr,
                        num_idxs_reg=sr,
                        elem_size=D,
                        single_packet=(c == 0 or s != 0),
                        queue_num=_gq[0] % 4,
                    )
                    _gq[0] += 1

                # ---- scores: fused multiply + free-axis reduce ----
                for i in range(CHUNK_TILES):
                    prod = prodp.tile([128, D], DT, tag="prod")
                    nc.vector.scalar_tensor_tensor(
                        out=prod[:],
                        in0=ne_tile(i),
                        scalar=1.0,
                        in1=wnb_sb[:],
                        op0=mybir.AluOpType.mult,
                        op1=mybir.AluOpType.mult,
                        accum_out=s_all[:, c * CHUNK_TILES + i : c * CHUNK_TILES + i + 1],
                    )

                if STAGE < 2:
                    if c == N_CHUNKS - 1:
                        nc.sync.dma_start(out=out_d[0:128, :], in_=s_all[:])
                    continue

                if c == 0:
                    # c_T0[j, g] = node_e[4j+g] . w_self  (fc_b folded into u)
                    for g in range(4):
                        prod = prodp.tile([128, D], DT, tag="prod")
                        nc.vector.scalar_tensor_tensor(
                            out=prod[:],
                            in0=node_sb[:, g, :],
                            scalar=1.0,
                            in1=wself_sb[:],
                            op0=mybir.AluOpType.mult,
                            op1=mybir.AluOpType.mult,
                            accum_out=c_T0[:, g : g + 1],
                        )

                # ---- transpose scores: [128, 32] -> [32, 128] ----
                sT_ps = psum_t.tile([32, 128], F32, tag="sT")
                nc.tensor.transpose(
                    out=sT_ps[:],
                    in_=s_all[:, c * CHUNK_TILES : (c + 1) * CHUNK_TILES],
                    identity=ident_sb[:],
                )

                # ---- softmax over k in transposed layout ----
                # u = scores_T + c_T (bias constant over k, varies per group)
                cslice = c_T0[32 * jb : 32 * jb + 32, :]
                u = smaxp.tile([32, 128], F32, tag="u")
                nc.vector.scalar_tensor_tensor(
                    out=u[:].rearrange("p (g k) -> p g k", g=4),
                    in0=sT_ps[:].rearrange("p (g k) -> p g k", g=4),
                    scalar=fcb,
                    in1=cslice.to_broadcast([32, 4, K]),
                    op0=mybir.AluOpType.add,
                    op1=mybir.AluOpType.add,
                )
                # leaky_relu(u) = max(0.2*u, u)
                lr = smaxp.tile([32, 128], F32, tag="lr")
                nc.vector.scalar_tensor_tensor(
                    out=lr[:],
                    in0=u[:],
                    scalar=NEG_SLOPE,
                    in1=u[:],
                    op0=mybir.AluOpType.mult,
                    op1=mybir.AluOpType.max,
                )
                ex = smaxp.tile([32, 128], F32, tag="ex")
                nc.scalar.activation(
                    out=ex[:],
                    in_=lr[:],
                    func=mybir.ActivationFunctionType.Exp,
                )
                dn = smaxp.tile([32, 4], F32, tag="dn")
                nc.vector.tensor_reduce(
                    out=dn[:],
                    in_=ex[:].rearrange("p (g k) -> p g k", g=4),
                    axis=mybir.AxisListType.X,
                    op=mybir.AluOpType.add,
                )
                rcp = smaxp.tile([32, 4], F32, tag="rcp")
                nc.vector.reciprocal(out=rcp[:], in_=dn[:])
                attn_T = smaxp.tile([32, 128], F32, tag="attn_T")
                attn_eng = nc.vector
                attn_eng.tensor_tensor(
                    out=attn_T[:].rearrange("p (g k) -> p g k", g=4),
                    in0=ex[:].rearrange("p (g k) -> p g k", g=4),
                    in1=rcp[:].to_broadcast([32, 4, K]),
                    op=mybir.AluOpType.mult,
                )

                # ---- transpose back: [32, 128] -> [128, 32] ----
                attn_ps = psum_t.tile([128, 32], F32, tag="attn_ps")
                nc.tensor.transpose(
                    out=attn_ps[:],
                    in_=attn_T[:],
                    identity=ident_sb[0:32, 0:32],
                )

                # ---- stationary matrices: am[p, j//8, j%8, m] =
                #      mask8[p, j%8, m] * attn[p, j] ----
                attn_sb = smaxp.tile([128, CHUNK_TILES], DT, tag="attn_sb")
                nc.scalar.copy(out=attn_sb[:], in_=attn_ps[:])
                am = smaxp.tile([128, 4, 8, 32], DT, tag="am")
                m_ap = mask_sb[:]
                mask_bc = bass.AP(
                    tensor=m_ap.tensor,
                    offset=m_ap.offset,
                    ap=[m_ap.ap[0], [0, 4], m_ap.ap[1], m_ap.ap[2]],
                )
                a_ap = attn_sb[:]
                attn_bc = bass.AP(
                    tensor=a_ap.tensor,
                    offset=a_ap.offset,
                    ap=[a_ap.ap[0], [8 * a_ap.ap[1][0], 4], [a_ap.ap[1][0], 8], [0, 32]],
                )
                am_eng = nc.gpsimd if c == 6 else nc.vector
                am_eng.tensor_tensor(
                    out=am[:],
                    in0=mask_bc,
                    in1=attn_bc,
                    op=mybir.AluOpType.mult,
                )

                if STAGE < 3:
                    if c == 0:
                        o32 = outp.tile([128, 32], F32, tag="o32")
                        nc.vector.tensor_copy(out=o32[:], in_=attn_sb[:])
                        nc.sync.dma_start(out=out_d[0:128, 0:32], in_=o32[:])
                    continue

                # ---- block-diagonal aggregation matmuls (M=32, 32-aligned) ----
                if c < 4:
                    agg = psum_agg.tile([128, D], F32, tag=f"agg{jb}")
                    _CACHE.setdefault("agg_tiles", {})[jb] = agg
                else:
                    agg = _CACHE["agg_tiles"][jb]
                for j in range(CHUNK_TILES):
                    qpos = 32 * (j // 8)
                    nc.tensor.matmul(
                        out=agg[qpos : qpos + 32, :],
                        lhsT=am[:, j // 8, j % 8, :],
                        rhs=ne_tile(j),
                        start=(c < 4 and j % 8 == 0),
                        stop=(c >= 4 and j % 8 == 7),
                        skip_group_check=True,
                        tile_position=(0, qpos),
                    )

                # ---- epilogue: out = agg + (S*K) * node_e ----
                if c >= 4:
                    o_sb = outp.tile([128, D], F32, tag="o_sb")
                    nc.vector.scalar_tensor_tensor(
                        out=o_sb[:],
                        in0=node_sb[:, 4 + jb, :],
                        scalar=float(STEPS * K),
                        in1=agg[:],
                        op0=mybir.AluOpType.mult,
                        op1=mybir.AluOpType.add,
                    )
                    nc.sync.dma_start(
                        out=out_d[128 * jb : 128 * (jb + 1), :], in_=o_sb[:]
                    )

    nc.compile()
    _CACHE.pop("agg_tiles", None)
    return nc


def _prep_core_inputs(core, node, neighbors, embeddings, npdt):
    """Host-side sharding: compact the table and remap indices (int16)."""
    node_c = np.asarray(node[B_LOC * core : B_LOC * (core + 1)])
    nb_c = np.asarray(neighbors[:, node_c, :])  # [S, B_LOC, K]
    flat = nb_c.reshape(-1).astype(np.int64)  # row r = s*B_LOC*K + b*K + k
    allidx = np.concatenate([flat, node_c.astype(np.int64)])
    uniq, inv = np.unique(allidx, return_inverse=True)
    U = len(uniq)
    assert U <= U_PAD, f"core {core}: {U} unique rows exceed {U_PAD}"
    tbl = np.zeros((U_PAD, D), dtype=npdt)
    tbl[:U] = embeddings[uniq].astype(npdt)

    flat16 = inv[:ROWS].astype(np.int16)
    node16 = inv[ROWS:].astype(np.int16)

    # neighbor indices, wrapped per chunk: index q of chunk c sits at
    # [partition q%16 (replicated x8), slot c*256 + q//16]
    ne_w = np.zeros((128, ROWS // 16), dtype=np.int16)
    for c in range(N_CHUNKS):
        chunk = flat16[CHUNK_ROWS * c : CHUNK_ROWS * (c + 1)]
        wrapped = chunk.reshape(CHUNK_ROWS // 16, 16).T  # [16, 256]
        ne_w[:, (ROWS // 16 // N_CHUNKS) * c : (ROWS // 16 // N_CHUNKS) * (c + 1)] = (
            np.tile(wrapped, (8, 1))
        )

    # node gathers: c-order (gathered row i -> node[4*(i%128) + i//128]),
    # then natural order
    i = np.arange(B_LOC)
    cidx = node16[4 * (i % 128) + i // 128]
    nd = np.concatenate([cidx, node16])  # 1024 indices
    nd_w = np.tile(nd.reshape(64, 16).T, (8, 1)).astype(np.int16)  # [128, 64]

    return {"table": tbl, "neidx": ne_w, "ndidx": nd_w}


def kernel(node, neighbors, embeddings, fc_w, fc_b, _trace=False):
    node = np.asarray(node)
    neighbors = np.asarray(neighbors)
    embeddings = np.asarray(embeddings, dtype=np.float32)
    fc_w = np.asarray(fc_w, dtype=np.float32)
    fc_b = np.asarray(fc_b, dtype=np.float32)

    npdt = _np_dt(DT_NAME)
    key = (DT_NAME, fc_w.tobytes(), fc_b.tobytes())
    if _CACHE.get("key") != key:
        _CACHE["nc"] = _build_nc(DT_NAME, fc_w, fc_b)
        _CACHE["key"] = key
    nc = _CACHE["nc"]

    in_maps = [
        _prep_core_inputs(c, node, neighbors, embeddings, npdt)
        for c in range(N_CORES)
    ]
    res = run_bass_kernel_spmd(
        nc, in_maps, core_ids=list(range(N_CORES)), trace=_trace
    )
    out = np.concatenate([res.results[c]["out"] for c in range(N_CORES)], axis=0)
    if _trace:
        _CACHE["last_exec_time_ns"] = res.exec_time_ns
        _CACHE["last_results"] = res
    return out


# revision 41
# speedup vs baseline: 1.2856x; 1.0036x over previous
"""Trainium2 Bass kernel for GNN attention message passing.

Reference computation (per query node b, step s, neighbors k=0..31):
    scores[s,b,k] = ne[s,b,k] . w_nb + node_e[b] . w_self + fc_b
    attn = softmax_k(leaky_relu(scores, 0.2))
    out[b] = sum_{s,k} attn[s,b,k] * ne[s,b,k] + S*K * node_e[b]

Sharding: data-parallel over the node batch B=4096 across 8 cores (512
query nodes per core).  Each core receives a compacted bf16 embedding
table holding each row it needs exactly once (host-side np.unique remap
so indices fit int16 for the on-device dma_gather) and gathers all
2*512*32 = 32768 neighbor rows on device.

Per-core pipeline (per 4096-row chunk, 8 chunks):
  * mixed-mode dma_gather: the 4 x 1024-row sub-gathers alternate
    per-descriptor-packet mode (cheap Q7 descriptor generation, drains
    on the 4 queue-bound DMA engines) and single-packet mode (pricier
    generation, drains across all 16 DMA engines), rotating over the 4
    SWDGE queues -- balancing the two per-descriptor bottlenecks gives
    ~120us for the gather stream vs ~206us for either mode alone
  * scores: fused multiply + free-axis-reduce (scalar_tensor_tensor
    with accum_out) on the vector engine, one op per 128-row tile
  * softmax runs in a transposed layout (TensorE transpose puts the
    tile index on partitions, neighbor index on the free axis) so the
    k=32 segments reduce on the free axis; fc_b + the node-term bias
    fold into one scalar_tensor_tensor; exp on the scalar engine
  * aggregation: block-diagonal M=32 matmuls on TensorE (stationary =
    position-mask * attn), accumulating both steps in 4 PSUM banks;
    epilogue adds (S*K) * node_e and streams results out

All engines overlap; measured ~157us/8-core-chip, rel err ~1.7e-3
(bf16 storage, fp32 accumulation).  KERNEL_DT=f32 gives an exact
(3e-8) fallback at ~300us.
"""

import os
import sys

for _p in ("/opt/trn_rl_repo", "/root/.axon_site/_ro/trn_rl_repo"):
    if os.path.isdir(_p) and _p not in sys.path:
        sys.path.insert(0, _p)

import numpy as np

import concourse.bass as bass
import concourse.bacc as bacc
import concourse.tile as tile
from concourse import mybir
from concourse.bass_utils import run_bass_kernel_spmd

# Problem constants (hardcoded per spec)
N_NODES = 100000
D = 256
STEPS = 2
K = 32
B = 4096
NEG_SLOPE = 0.2
N_CORES = 8

B_LOC = B // N_CORES  # 512 query nodes per core
ROWS = STEPS * B_LOC * K  # 32768 gathered neighbor rows per core
TILES = ROWS // 128  # 256
CHUNK_TILES = 32  # tiles per gather chunk
CHUNK_ROWS = CHUNK_TILES * 128  # 4096
N_CHUNKS = TILES // CHUNK_TILES  # 8
U_PAD = 32768  # compacted table rows (padded, fits int16 indexing)

# bf16 storage for the embedding table halves HBM traffic and doubles
# vector-engine throughput; fp32 accumulation throughout keeps the
# result well inside the 2e-2 relative-error gate.
DT_NAME = os.environ.get("KERNEL_DT", "bf16")

_CACHE = {}


def _np_dt(dt_name):
    if dt_name == "bf16":
        import ml_dtypes

        return np.dtype(ml_dtypes.bfloat16)
    return np.dtype(np.float32)


STAGE = int(os.environ.get("KERNEL_STAGE", "9"))  # 1=scores 2=softmax 9=full


def _build_nc(dt_name, fc_w, fc_b):
    """Build the per-core Bass graph (same NEFF for all 8 cores)."""
    DT = mybir.dt.bfloat16 if dt_name == "bf16" else mybir.dt.float32
    F32 = mybir.dt.float32
    npdt = _np_dt(dt_name)

    nc = bacc.Bacc(num_swdge_queues=4)

    table = nc.dram_tensor("table", [U_PAD, D], DT, kind="ExternalInput")
    neidx = nc.dram_tensor(
        "neidx", [128, ROWS // 16], mybir.dt.int16, kind="ExternalInput"
    )
    ndidx = nc.dram_tensor("ndidx", [128, 64], mybir.dt.int16, kind="ExternalInput")
    out_d = nc.dram_tensor("out", [B_LOC, D], F32, kind="ExternalOutput")

    w_nb = np.asarray(fc_w[0, :D], dtype=np.float32)
    w_self = np.asarray(fc_w[0, D:], dtype=np.float32)
    fcb = float(np.asarray(fc_b).reshape(-1)[0])

    wnb_c = nc.inline_tensor(
        np.tile(w_nb[None, :], (128, 1)).astype(npdt), name="wnb_c"
    )
    wself_c = nc.inline_tensor(
        np.tile(w_self[None, :], (128, 1)).astype(npdt), name="wself_c"
    )
    # mask8[p, q, m] = 1 iff m == 4q + p//32: selects the output column for
    # a tile at position q (of 8) within a 32-b output quarter
    mask8_np = np.zeros((128, 8, 32), dtype=np.float32)
    for p in range(128):
        for q in range(8):
            mask8_np[p, q, 4 * q + p // 32] = 1.0
    mask_c = nc.inline_tensor(mask8_np.astype(npdt), name="mask_c")
    ident_c = nc.inline_tensor(np.eye(128, dtype=np.float32), name="ident_c")

    with tile.TileContext(nc) as tc:
        with (
            tc.tile_pool(name="consts", bufs=1) as consts,
            tc.tile_pool(name="idxp", bufs=1) as idxp,
            tc.tile_pool(name="nep", bufs=6 if dt_name == "bf16" else 2) as nep,
            tc.tile_pool(name="prodp", bufs=8) as prodp,
            tc.tile_pool(name="scorep", bufs=1) as scorep,
            tc.tile_pool(name="smaxp", bufs=3) as smaxp,
            tc.tile_pool(name="outp", bufs=2) as outp,
            tc.tile_pool(name="psum_t", bufs=2, space="PSUM") as psum_t,
            tc.tile_pool(name="psum_agg", bufs=1, space="PSUM") as psum_agg,
        ):
            # ---- index tensors first (the chunk-0 gather is the critical path) ----
            neidx_sb = idxp.tile([128, ROWS // 16], mybir.dt.int16, tag="neidx")
            _slot = CHUNK_ROWS // 16
            nc.sync.dma_start(out=neidx_sb[:, 0:_slot], in_=neidx[:, 0:_slot])
            ndidx_sb = idxp.tile([128, 64], mybir.dt.int16, tag="ndidx")
            nc.sync.dma_start(out=ndidx_sb[:], in_=ndidx[:])
            for _c in range(1, N_CHUNKS):
                nc.sync.dma_start(
                    out=neidx_sb[:, _c * _slot : (_c + 1) * _slot],
                    in_=neidx[:, _c * _slot : (_c + 1) * _slot],
                )

            # ---- constants to SBUF (ACT HWDGE ring; not on the gather critical path) ----
            wnb_sb = consts.tile([128, D], DT, tag="wnb")
            nc.scalar.dma_start(out=wnb_sb[:], in_=wnb_c[:])
            wself_sb = consts.tile([128, D], DT, tag="wself")
            nc.scalar.dma_start(out=wself_sb[:], in_=wself_c[:])
            mask_sb = consts.tile([128, 8, 32], DT, tag="mask")
            nc.scalar.dma_start(out=mask_sb[:], in_=mask_c[:])
            ident_sb = consts.tile([128, 128], F32, tag="ident")
            nc.scalar.dma_start(out=ident_sb[:], in_=ident_c[:])

            s_all = scorep.tile([128, TILES], F32, tag="s_all")
            _gq = [0]
            node_sb = consts.tile([128, 8, D], DT, tag="node_sb")
            c_T0 = consts.tile([128, 4], F32, tag="c_T0")

            for c in range(N_CHUNKS):
                jb = c % 4
                # ---- gather 4096 neighbor embedding rows ----
                nslots = CHUNK_ROWS // 16
                nsub = 4
                stiles = CHUNK_TILES // nsub
                ne_subs = [
                    nep.tile(
                        [128, stiles, D], DT,
                        tag=f"ne{s}", name=f"ne_c{c}s{s}",
                    )
                    for s in range(nsub)
                ]

                def ne_tile(i, _subs=ne_subs, _st=stiles):
                    return _subs[i // _st][:, i % _st, :]

                # mixed-mode gather: sub-gather 0 uses per-descriptor packets
                # (cheap descriptor generation, drains on the 4 queue-bound
                # engines); sub-gathers 1-3 use single-packet mode (pricier
                # generation, drains across all 16 DMA engines).  Interleaving
                # the two balances the Q7 generation and engine-drain limits.
                for s in range(nsub):
                    sr = CHUNK_ROWS // nsub
                    ss = nslots // nsub
                    if c == 0 and s == 0:
                        # node-embedding rows first (small; unblocks the score
                        # bias c_T0), then chunk 0's first neighbor sub-gather
                        # single-packet: spreads across all 16 DMA engines so
                        # the very first tiles' data lands as early as possible
                        nc.gpsimd.dma_gather(
                            out_ap=ne_subs[0][:],
                            in_ap=table[:],
                            idxs_ap=neidx_sb[:, 0:ss],
                            num_idxs=sr,
                            num_idxs_reg=sr,
                            elem_size=D,
                            single_packet=True,
                            queue_num=0,
                        )
                        nc.gpsimd.dma_gather(
                            out_ap=node_sb[:],
                            in_ap=table[:],
                            idxs_ap=ndidx_sb[:],
                            num_idxs=2 * B_LOC,
                            num_idxs_reg=2 * B_LOC,
                            elem_size=D,
                            single_packet=False,
                            queue_num=1,
                        )
                        _gq[0] += 2
                        continue
                    nc.gpsimd.dma_gather(
                        out_ap=ne_subs[s][:],
                        in_ap=table[:],
                        idxs_ap=neidx_sb[:, c * nslots + s * ss : c * nslots + (s + 1) * ss],
                        num_idxs=sr,
                        num_idxs_reg=sr,
                        elem_size=D,
                        single_packet=(c == 0 or s != 0),
                        queue_num=_gq[0] % 4,
                    )
                    _gq[0] += 1

                # ---- scores: fused multiply + free-axis reduce ----
                for i in range(CHUNK_TILES):
                    prod = prodp.tile([128, D], DT, tag="prod")
                    nc.vector.scalar_tensor_tensor(
                        out=prod[:],
                        in0=ne_tile(i),
                        scalar=1.0,
                        in1=wnb_sb[:],
                        op0=mybir.AluOpType.mult,
                        op1=mybir.AluOpType.mult,
                        accum_out=s_all[:, c * CHUNK_TILES + i : c * CHUNK_TILES + i + 1],
                    )

                if STAGE < 2:
                    if c == N_CHUNKS - 1:
                        nc.sync.dma_start(out=out_d[0:128, :], in_=s_all[:])
                    continue

                if c == 0:
                    # c_T0[j, g] = node_e[4j+g] . w_self  (fc_b folded into u)
                    for g in range(4):
                        prod = prodp.tile([128, D], DT, tag="prod")
                        nc.vector.scalar_tensor_tensor(
                            out=prod[:],
                            in0=node_sb[:, g, :],
                            scalar=1.0,
                            in1=wself_sb[:],
                            op0=mybir.AluOpType.mult,
                            op1=mybir.AluOpType.mult,
                            accum_out=c_T0[:, g : g + 1],
                        )

                # ---- transpose scores: [128, 32] -> [32, 128] ----
                sT_ps = psum_t.tile([32, 128], F32, tag="sT")
                nc.tensor.transpose(
                    out=sT_ps[:],
                    in_=s_all[:, c * CHUNK_TILES : (c + 1) * CHUNK_TILES],
                    identity=ident_sb[:],
                )

                # ---- softmax over k in transposed layout ----
                # u = scores_T + c_T (bias constant over k, varies per group)
                cslice = c_T0[32 * jb : 32 * jb + 32, :]
                u = smaxp.tile([32, 128], F32, tag="u")
                nc.vector.scalar_tensor_tensor(
                    out=u[:].rearrange("p (g k) -> p g k", g=4),
                    in0=sT_ps[:].rearrange("p (g k) -> p g k", g=4),
                    scalar=fcb,
                    in1=cslice.to_broadcast([32, 4, K]),
                    op0=mybir.AluOpType.add,
                    op1=mybir.AluOpType.add,
                )
                # leaky_relu(u) = max(0.2*u, u)
                lr = smaxp.tile([32, 128], F32, tag="lr")
                nc.vector.scalar_tensor_tensor(
                    out=lr[:],
                    in0=u[:],
                    scalar=NEG_SLOPE,
                    in1=u[:],
                    op0=mybir.AluOpType.mult,
                    op1=mybir.AluOpType.max,
                )
                ex = smaxp.tile([32, 128], F32, tag="ex")
                nc.scalar.activation(
                    out=ex[:],
                    in_=lr[:],
                    func=mybir.ActivationFunctionType.Exp,
                )
                dn = smaxp.tile([32, 4], F32, tag="dn")
                nc.vector.tensor_reduce(
                    out=dn[:],
                    in_=ex[:].rearrange("p (g k) -> p g k", g=4),
                    axis=mybir.AxisListType.X,
                    op=mybir.AluOpType.add,
                )
                rcp = smaxp.tile([32, 4], F32, tag="rcp")
                nc.vector.reciprocal(out=rcp[:], in_=dn[:])
                attn_T = smaxp.tile([32, 128], F32, tag="attn_T")
                attn_eng = nc.vector
                attn_eng.tensor_tensor(
                    out=attn_T[:].rearrange("p (g k) -> p g k", g=4),
                    in0=ex[:].rearrange("p (g k) -> p g k", g=4),
                    in1=rcp[:].to_broadcast([32, 4, K]),
                    op=mybir.AluOpType.mult,
                )

                # ---- transpose back: [32, 128] -> [128, 32] ----
                attn_ps = psum_t.tile([128, 32], F32, tag="attn_ps")
                nc.tensor.transpose(
                    out=attn_ps[:],
                    in_=attn_T[:],
                    identity=ident_sb[0:32, 0:32],
                )

                # ---- stationary matrices: am[p, j//8, j%8, m] =
                #      mask8[p, j%8, m] * attn[p, j] ----
                attn_sb = smaxp.tile([128, CHUNK_TILES], DT, tag="attn_sb")
                nc.scalar.copy(out=attn_sb[:], in_=attn_ps[:])
                am = smaxp.tile([128, 4, 8, 32], DT, tag="am")
                m_ap = mask_sb[:]
                mask_bc = bass.AP(
                    tensor=m_ap.tensor,
                    offset=m_ap.offset,
                    ap=[m_ap.ap[0], [0, 4], m_ap.ap[1], m_ap.ap[2]],
                )
                a_ap = attn_sb[:]
                attn_bc = bass.AP(
                    tensor=a_ap.tensor,
                    offset=a_ap.offset,
                    ap=[a_ap.ap[0], [8 * a_ap.ap[1][0], 4], [a_ap.ap[1][0], 8], [0, 32]],
                )
                am_eng = nc.gpsimd if c == 6 else nc.vector
                am_eng.tensor_tensor(
                    out=am[:],
                    in0=mask_bc,
                    in1=attn_bc,
                    op=mybir.AluOpType.mult,
                )

                if STAGE < 3:
                    if c == 0:
                        o32 = outp.tile([128, 32], F32, tag="o32")
                        nc.vector.tensor_copy(out=o32[:], in_=attn_sb[:])
                        nc.sync.dma_start(out=out_d[0:128, 0:32], in_=o32[:])
                    continue

                # ---- block-diagonal aggregation matmuls (M=32, 32-aligned) ----
                if c < 4:
                    agg = psum_agg.tile([128, D], F32, tag=f"agg{jb}")
                    _CACHE.setdefault("agg_tiles", {})[jb] = agg
                else:
                    agg = _CACHE["agg_tiles"][jb]
                for j in range(CHUNK_TILES):
                    qpos = 32 * (j // 8)
                    nc.tensor.matmul(
                        out=agg[qpos : qpos + 32, :],
                        lhsT=am[:, j // 8, j % 8, :],
                        rhs=ne_tile(j),
                        start=(c < 4 and j % 8 == 0),
                        stop=(c >= 4 and j % 8 == 7),
                        skip_group_check=True,
                        tile_position=(0, qpos),
                    )

                # ---- epilogue: out = agg + (S*K) * node_e ----
                if c >= 4:
                    o_sb = outp.tile([128, D], F32, tag="o_sb")
                    nc.vector.scalar_tensor_tensor(
                        out=o_sb[:],
                        in0=node_sb[:, 4 + jb, :],
                        scalar=float(STEPS * K),
                        in1=agg[:],
                        op0=mybir.AluOpType.mult,
                        op1=mybir.AluOpType.add,
                    )
                    nc.sync.dma_start(
                        out=out_d[128 * jb : 128 * (jb + 1), :], in_=o_sb[:]
                    )

    nc.compile()
    _CACHE.pop("agg_tiles", None)
    return nc


def _prep_core_inputs(core, node, neighbors, embeddings, npdt):
    """Host-side sharding: compact the table and remap indices (int16)."""
    node_c = np.asarray(node[B_LOC * core : B_LOC * (core + 1)])
    nb_c = np.asarray(neighbors[:, node_c, :])  # [S, B_LOC, K]
    flat = nb_c.reshape(-1).astype(np.int64)  # row r = s*B_LOC*K + b*K + k
    allidx = np.concatenate([flat, node_c.astype(np.int64)])
    uniq, inv = np.unique(allidx, return_inverse=True)
    U = len(uniq)
    assert U <= U_PAD, f"core {core}: {U} unique rows exceed {U_PAD}"
    tbl = np.zeros((U_PAD, D), dtype=npdt)
    tbl[:U] = embeddings[uniq].astype(npdt)

    flat16 = inv[:ROWS].astype(np.int16)
    node16 = inv[ROWS:].astype(np.int16)

    # neighbor indices, wrapped per chunk: index q of chunk c sits at
    # [partition q%16 (replicated x8), slot c*256 + q//16]
    ne_w = np.zeros((128, ROWS // 16), dtype=np.int16)
    for c in range(N_CHUNKS):
        chunk = flat16[CHUNK_ROWS * c : CHUNK_ROWS * (c + 1)]
        wrapped = chunk.reshape(CHUNK_ROWS // 16, 16).T  # [16, 256]
        ne_w[:, (ROWS // 16 // N_CHUNKS) * c : (ROWS // 16 // N_CHUNKS) * (c + 1)] = (
            np.tile(wrapped, (8, 1))
        )

    # node gathers: c-order (gathered row i -> node[4*(i%128) + i//128]),
    # then natural order
    i = np.arange(B_LOC)
    cidx = node16[4 * (i % 128) + i // 128]
    nd = np.concatenate([cidx, node16])  # 1024 indices
    nd_w = np.tile(nd.reshape(64, 16).T, (8, 1)).astype(np.int16)  # [128, 64]

    return {"table": tbl, "neidx": ne_w, "ndidx": nd_w}


def kernel(node, neighbors, embeddings, fc_w, fc_b, _trace=False):
    node = np.asarray(node)
    neighbors = np.asarray(neighbors)
    embeddings = np.asarray(embeddings, dtype=np.float32)
    fc_w = np.asarray(fc_w, dtype=np.float32)
    fc_b = np.asarray(fc_b, dtype=np.float32)

    npdt = _np_dt(DT_NAME)
    key = (DT_NAME, fc_w.tobytes(), fc_b.tobytes())
    if _CACHE.get("key") != key:
        _CACHE["nc"] = _build_nc(DT_NAME, fc_w, fc_b)
        _CACHE["key"] = key
    nc = _CACHE["nc"]

    in_maps = [
        _prep_core_inputs(c, node, neighbors, embeddings, npdt)
        for c in range(N_CORES)
    ]
    res = run_bass_kernel_spmd(
        nc, in_maps, core_ids=list(range(N_CORES)), trace=_trace
    )
    out = np.concatenate([res.results[c]["out"] for c in range(N_CORES)], axis=0)
    if _trace:
        _CACHE["last_exec_time_ns"] = res.exec_time_ns
        _CACHE["last_results"] = res
    return out
